# revision 1
# baseline (speedup 1.0000x reference)
"""Trainium2 Bass kernel for nn_CosSimConv2D.

Math (per sample b):
  s    = im2col3x3(x) @ w_hat           where w_hat = w / (||w||_col + qv)
  out  = sign(s) * exp(a_u/2 * (ln(s^2) - ln(box)))
  box  = 3x3 box-filter of per-pixel sum(x^2)  (= ||im2col row||^2)
  a    = softmax(p)
(The eps=1e-12 terms of the reference are dropped; they are ~1e-7-relative.)

GEMM precision: x and w_hat are each split hi+lo in bf16; three product
terms (xh@wh + xh@wl + xl@wh) recover ~fp32-grade dot products.
Data-parallel over batch: core b computes sample b.

Layouts on device (per core):
  alloc1 (128p, 130*130) bf16 : partitions 0-63 = x_hi^T padded image,
                                partitions 64-127 = x_lo^T padded image
  alloc2 (128p, 130*130) bf16 : partitions 0-63 = x_hi^T,
                                partitions 64-127 = x_hi^T shifted +1 col
  out tiles: (128 units, 512 pixels) in PSUM -> epilogue -> DRAM (128u, 16384pix)
Host transposes the per-core result back to (H, W, UNITS).
"""

import sys

sys.path.insert(0, "/opt/trn_rl_repo")

import numpy as np
import ml_dtypes

import concourse.bass as bass
import concourse.mybir as mybir
import concourse.tile as tile
from concourse import bacc
from concourse.bass_utils import run_bass_kernel_spmd
from concourse.masks import make_identity

BF16 = mybir.dt.bfloat16
F32 = mybir.dt.float32
AF = mybir.ActivationFunctionType

B, H, W, C, UNITS = 8, 128, 128, 64, 128
HW = H * W  # 16384
HP, WP = H + 2, W + 2  # 130x130 padded image
NTAP = 9
SLAB = 16  # image rows per streaming slab
NSLAB = H // SLAB
TILE_ROWS = 4  # image rows per output tile -> N = 512
NT = H // TILE_ROWS  # 32 output tiles
NPIX = TILE_ROWS * W  # 512

_CACHE = {}


def _build():
    nc = bacc.Bacc("TRN2", target_bir_lowering=False, debug=False)

    x_d = nc.dram_tensor("x", [HW, C], F32, kind="ExternalInput")
    wt13_d = nc.dram_tensor("wt13", [NTAP, 128, UNITS], BF16, kind="ExternalInput")
    wt2p_d = nc.dram_tensor("wt2p", [3, 128, UNITS], BF16, kind="ExternalInput")
    ws_last_d = nc.dram_tensor("ws_last", [3, 64, UNITS], BF16, kind="ExternalInput")
    a2_d = nc.dram_tensor("a2", [128, 1], F32, kind="ExternalInput")
    band_d = nc.dram_tensor("band", [128, 128], BF16, kind="ExternalInput")
    out_d = nc.dram_tensor("out", [128, HW], F32, kind="ExternalOutput")

    with tile.TileContext(nc) as tc:
        with (
            tc.tile_pool(name="const", bufs=1) as constp,
            tc.tile_pool(name="big", bufs=1) as bigp,
            tc.tile_pool(name="slab", bufs=2) as slabp,
            tc.tile_pool(name="epi", bufs=3) as epip,
            tc.tile_pool(name="ptr", bufs=2, space="PSUM") as ptrp,
            tc.tile_pool(name="pmm", bufs=2, space="PSUM") as pmmp,
            tc.tile_pool(name="pmisc", bufs=1, space="PSUM") as pmiscp,
        ):
            # ---- constants ----
            wt13 = constp.tile([128, NTAP, UNITS], BF16, tag="wt13")
            nc.sync.dma_start(out=wt13, in_=wt13_d.ap().rearrange("t k u -> k t u"))
            wt2p = constp.tile([128, 3, UNITS], BF16, tag="wt2p")
            nc.sync.dma_start(out=wt2p, in_=wt2p_d.ap().rearrange("t k u -> k t u"))
            ws_last = constp.tile([64, 3, UNITS], BF16, tag="wsl")
            nc.sync.dma_start(out=ws_last, in_=ws_last_d.ap().rearrange("t k u -> k t u"))
            a2 = constp.tile([128, 1], F32, tag="a2")
            nc.sync.dma_start(out=a2, in_=a2_d[:, :])
            band = constp.tile([128, 128], BF16, tag="band")
            nc.sync.dma_start(out=band, in_=band_d[:, :])
            ident = constp.tile([128, 128], BF16, tag="ident")
            make_identity(nc, ident)

            # ---- big persistent buffers ----
            alloc1 = bigp.tile([128, HP * WP], BF16, tag="alloc1")
            alloc2 = bigp.tile([128, HP * WP], BF16, tag="alloc2")
            a1v = alloc1.rearrange("p (hp wp) -> p hp wp", wp=WP)
            a2v = alloc2.rearrange("p (hp wp) -> p hp wp", wp=WP)
            lbc = bigp.tile([128, HW], BF16, tag="lbc")
            s2p = bigp.tile([128, HP], BF16, tag="s2p")  # (w, padded h)

            # zero borders of alloc1/alloc2: rows hp=0,129 and cols wp=0,129
            for av in (a1v, a2v):
                nc.vector.memset(av[:, 0, :], 0.0)
                nc.vector.memset(av[:, HP - 1, :], 0.0)
                nc.vector.memset(av[:, :, 0], 0.0)
                nc.vector.memset(av[:, :, WP - 1], 0.0)
            nc.vector.memset(s2p[:, 0:1], 0.0)
            nc.vector.memset(s2p[:, HP - 1 : HP], 0.0)

            xv = x_d.ap().rearrange("(h w) c -> h w c", w=W)

            # ---- pass 1: sum-of-squares image for norms (all slabs) ----
            for s in range(NSLAB):
                h0 = s * SLAB
                xnat = slabp.tile([128, SLAB, C], F32, tag="xnata")
                nc.sync.dma_start(
                    out=xnat, in_=xv[h0 : h0 + SLAB].rearrange("h w c -> w h c")
                )
                xsq = slabp.tile([128, SLAB, C], BF16, tag="xsq")
                nc.scalar.activation(out=xsq, in_=xnat, func=AF.Square)
                with nc.allow_low_precision(reason="s2 bf16 ~5e-4 rel; out err ~a*2.5e-4"):
                    nc.vector.tensor_reduce(
                        out=s2p[:, 1 + h0 : 1 + h0 + SLAB],
                        in_=xsq,
                        axis=mybir.AxisListType.X,
                        op=mybir.AluOpType.add,
                    )

            # ---- norm image: box filter + log + broadcast ----
            timg = bigp.tile([128, 128], BF16, tag="timg")  # (w, h) h-boxed
            nc.vector.tensor_tensor(
                out=timg, in0=s2p[:, 0:128], in1=s2p[:, 1:129], op=mybir.AluOpType.add
            )
            nc.vector.tensor_tensor(
                out=timg, in0=timg, in1=s2p[:, 2:130], op=mybir.AluOpType.add
            )
            boxp = pmiscp.tile([128, 128], F32, tag="boxp")
            nc.tensor.matmul(boxp, band, timg, start=True, stop=True)
            lpos = bigp.tile([128, 128], BF16, tag="lpos")
            nc.scalar.activation(out=lpos, in_=boxp, func=AF.Ln)
            lneg = bigp.tile([128, 128], BF16, tag="lneg")
            nc.vector.tensor_scalar_mul(out=lneg, in0=lpos, scalar1=-1.0)
            ltp = pmiscp.tile([128, 128], BF16, tag="ltp")
            nc.tensor.transpose(ltp, lneg, ident)
            lrow = bigp.tile([128, 128], BF16, tag="lrow")  # (h, w)
            nc.vector.tensor_copy(out=lrow, in_=ltp)
            nc.sync.dma_start(out=lbc[0:1, :], in_=lrow)
            n = 1
            while n < 128:
                nc.sync.dma_start(out=lbc[n : 2 * n, :], in_=lbc[0:n, :])
                n *= 2

            # ---- pass 2: transposes interleaved with GEMM tiles ----
            def slab_prep(s):
                h0 = s * SLAB
                xnat = slabp.tile([128, SLAB, C], F32, tag="xnatb")
                nc.sync.dma_start(
                    out=xnat, in_=xv[h0 : h0 + SLAB].rearrange("h w c -> w h c")
                )
                packed = slabp.tile([128, SLAB, 2, C], BF16, tag="packed")
                nc.vector.tensor_copy(out=packed[:, :, 0, :], in_=xnat)
                nc.vector.tensor_tensor(
                    out=packed[:, :, 1, :],
                    in0=xnat,
                    in1=packed[:, :, 0, :],
                    op=mybir.AluOpType.subtract,
                )
                for g in range(SLAB // 8):
                    ptr = ptrp.tile([128, 8, 128], BF16, tag="ptr")
                    for r in range(8):
                        hl = g * 8 + r
                        nc.tensor.transpose(
                            ptr[:, r, :],
                            packed[:, hl, :, :].rearrange("p t c -> p (t c)"),
                            ident,
                        )
                    hp0 = h0 + g * 8 + 1
                    nc.vector.tensor_copy(out=a1v[:, hp0 : hp0 + 8, 1 : 1 + W], in_=ptr)
                    nc.sync.dma_start(
                        out=a2v[0:64, hp0 : hp0 + 8, :],
                        in_=a1v[0:64, hp0 : hp0 + 8, :],
                    )
                    nc.sync.dma_start(
                        out=alloc2.rearrange("p (hp wp) -> p hp wp", wp=WP)[
                            64:128, hp0 : hp0 + 8, 0 : WP - 1
                        ],
                        in_=alloc1.rearrange("p (hp wp) -> p hp wp", wp=WP)[
                            0:64, hp0 : hp0 + 8, 1:WP
                        ],
                    )

            # ---- GEMM + epilogue per output tile ----
            def emit_tile(j):
                hh = j * TILE_ROWS
                ps = pmmp.tile([128, TILE_ROWS, W], F32, tag="ps")
                first = True
                for ty in range(3):
                    for tx in range(3):
                        nc.tensor.matmul(
                            ps,
                            wt13[:, ty * 3 + tx, :],
                            a1v[:, hh + ty : hh + ty + TILE_ROWS, tx : tx + W],
                            start=first,
                            stop=False,
                        )
                        first = False
                for ty in range(3):
                    nc.tensor.matmul(
                        ps,
                        wt2p[:, ty, :],
                        a2v[:, hh + ty : hh + ty + TILE_ROWS, 0:W],
                        start=False,
                        stop=False,
                    )
                for ty in range(3):
                    nc.tensor.matmul(
                        ps,
                        ws_last[:, ty, :],
                        a2v[0:64, hh + ty : hh + ty + TILE_ROWS, 2 : 2 + W],
                        start=False,
                        stop=(ty == 2),
                    )
                psf = ps.rearrange("p r w -> p (r w)")
                sq = epip.tile([128, NPIX], BF16, tag="sq")
                nc.scalar.activation(out=sq, in_=psf, func=AF.Square)
                sgn = epip.tile([128, NPIX], F32, tag="sgn")
                nc.scalar.activation(out=sgn, in_=psf, func=AF.Sign)
                v = epip.tile([128, NPIX], BF16, tag="v")
                nc.scalar.activation(out=v, in_=sq, func=AF.Ln)
                v2 = epip.tile([128, NPIX], BF16, tag="v2")
                nc.vector.tensor_tensor(
                    out=v2,
                    in0=v,
                    in1=lbc[:, j * NPIX : (j + 1) * NPIX],
                    op=mybir.AluOpType.add,
                )
                t3 = epip.tile([128, NPIX], F32, tag="t3")
                nc.scalar.activation(out=t3, in_=v2, func=AF.Exp, scale=a2[:, :])
                o = epip.tile([128, NPIX], F32, tag="o")
                nc.vector.tensor_tensor(
                    out=o, in0=t3, in1=sgn, op=mybir.AluOpType.mult
                )
                nc.sync.dma_start(out=out_d[:, j * NPIX : (j + 1) * NPIX], in_=o)

            emitted = 0
            for s in range(NSLAB):
                slab_prep(s)
                while emitted <= min(4 * s + 2, NT - 1):
                    emit_tile(emitted)
                    emitted += 1
            while emitted < NT:
                emit_tile(emitted)
                emitted += 1

    nc.compile()
    return nc


def _host_prep(w, p, q):
    EPS = 1e-12
    w64 = w[0].astype(np.float64)  # (576, 128)
    qv = (q.astype(np.float64) ** 2 / 10.0)[0]
    wn = np.sqrt(np.maximum((w64**2).sum(0), EPS)) + qv
    what = (w64 / wn).astype(np.float32)
    wh = what.astype(ml_dtypes.bfloat16)
    wl = (what - wh.astype(np.float32)).astype(ml_dtypes.bfloat16)

    def tap(a, k):
        return np.ascontiguousarray(a[k * 64 : (k + 1) * 64, :])

    wt13 = np.stack([np.vstack([tap(wh, k), tap(wh, k)]) for k in range(9)])
    wt2p = np.stack(
        [np.vstack([tap(wl, 3 * ty + 0), tap(wl, 3 * ty + 1)]) for ty in range(3)]
    )
    ws_last = np.stack([tap(wl, 2), tap(wl, 5), tap(wl, 8)])

    pe = np.exp(p.astype(np.float64) - p.astype(np.float64).max())
    a = pe / pe.sum()
    a2 = (a * 0.5).astype(np.float32).reshape(128, 1)

    band = np.zeros((128, 128), dtype=np.float32)
    for i in range(128):
        band[i, max(0, i - 1) : i + 2] = 1.0
    band = band.astype(ml_dtypes.bfloat16)
    return wt13, wt2p, ws_last, a2, band


LAST_RESULTS = None


def kernel(inputs, w, p, q):
    global LAST_RESULTS
    if "nc" not in _CACHE:
        _CACHE["nc"] = _build()
    nc = _CACHE["nc"]

    wt13, wt2p, ws_last, a2, band = _host_prep(w, p, q)
    xs = np.ascontiguousarray(inputs.reshape(B, HW, C).astype(np.float32))
    in_maps = [
        {
            "x": xs[b],
            "wt13": wt13,
            "wt2p": wt2p,
            "ws_last": ws_last,
            "a2": a2,
            "band": band,
        }
        for b in range(B)
    ]
    import os

    trace = bool(int(os.environ.get("KERNEL_TRACE", "0")))
    res = run_bass_kernel_spmd(nc, in_maps, core_ids=list(range(B)), trace=trace)
    LAST_RESULTS = res
    out = np.stack(
        [res.results[b]["out"].T.reshape(H, W, UNITS) for b in range(B)]
    ).astype(np.float32)
    return out



# revision 13
# speedup vs baseline: 1.2864x; 1.2864x over previous
"""Trainium2 Bass kernel for nn_CosSimConv2D.

Math (per sample b):
  s    = im2col3x3(x) @ w_hat           where w_hat = w / (||w||_col + qv)
  out  = sign(s) * exp(a_u/2 * (ln(s^2) - ln(box)))
  box  = 3x3 box-filter of per-pixel sum(x^2)  (= ||im2col row||^2)
  a    = softmax(p)
(The eps=1e-12 terms of the reference are dropped; they are ~1e-7-relative.)

GEMM precision: x and w_hat are each split hi+lo in bf16; three product
terms (xh@wh + xl@wh + xh@wl) recover ~fp32-grade dot products.
Data-parallel over batch: core b computes sample b.

Device layouts (host-prepared, DMA'd straight into SBUF):
  A1 (128p, 130*130) bf16 : p0-63  = x_hi^T zero-padded image,
                            p64-127 = x_lo^T zero-padded image
  A2 (128p, 130*130) bf16 : p0-63  = x_hi^T,
                            p64-127 = x_hi^T shifted +1 col
  XW (128p, 128*64)  bf16 : (w, h, c) layout of x_hi for the norm image
GEMM: 8 groups x 4 tiles x 15 matmuls (N=512) accumulate in a 4-bank
PSUM group tile (bufs=2 -> all 8 banks); epilogue per 2-group batch:
  sq = s*s (DVE), v = Ln(sq) (ACT), v2 = v - lnbox_bcast (DVE),
  t3 = Exp(a/2 * v2) (ACT), out = t3 | signbit(s) (DVE), DMA out bf16.
Host converts the (128u, HW) bf16 result to (H, W, UNITS) fp32.
"""

import sys

sys.path.insert(0, "/opt/trn_rl_repo")

import numpy as np
import ml_dtypes

import concourse.bass as bass
import concourse.mybir as mybir
import concourse.tile as tile
from concourse import bacc
from concourse.bass_utils import run_bass_kernel_spmd

BF16 = mybir.dt.bfloat16
F32 = mybir.dt.float32
AF = mybir.ActivationFunctionType
ALU = mybir.AluOpType

B, H, W, C, UNITS = 8, 128, 128, 64, 128
HW = H * W  # 16384
HP, WP = H + 2, W + 2  # 130x130 padded image
NTAP = 9
TILE_ROWS = 4  # image rows per output tile -> N = 512
NPIX = TILE_ROWS * W  # 512
GT = 4  # tiles per PSUM group (4 banks)
NG = 8  # groups
GPIX = GT * NPIX  # 2048
BG = 2  # groups per epilogue batch
NB = NG // BG  # 4 batches
BPIX = BG * GPIX  # 4096
PSLAB = 32  # image rows per norm-pass slab
NPSLAB = H // PSLAB  # 4

_CACHE = {}


def _build():
    nc = bacc.Bacc("TRN2", target_bir_lowering=False, debug=False)

    a1_d = nc.dram_tensor("a1", [128, HP * WP], BF16, kind="ExternalInput")
    a2_d = nc.dram_tensor("a2i", [128, HP * WP], BF16, kind="ExternalInput")
    xw_d = nc.dram_tensor("xw", [128, H * C], BF16, kind="ExternalInput")
    wt13_d = nc.dram_tensor("wt13", [NTAP, 128, UNITS], BF16, kind="ExternalInput")
    wt2p_d = nc.dram_tensor("wt2p", [3, 128, UNITS], BF16, kind="ExternalInput")
    wsl_d = nc.dram_tensor("wsl", [3, 64, UNITS], BF16, kind="ExternalInput")
    aexp_d = nc.dram_tensor("aexp", [128, 1], F32, kind="ExternalInput")
    out_d = nc.dram_tensor("out", [128, HW], BF16, kind="ExternalOutput")

    with tile.TileContext(nc) as tc:
        with (
            tc.tile_pool(name="const", bufs=1) as constp,
            tc.tile_pool(name="big", bufs=1) as bigp,
            tc.tile_pool(name="p1", bufs=2) as p1p,
            tc.tile_pool(name="epi", bufs=2) as epip,
            tc.tile_pool(name="pmm", bufs=2, space="PSUM") as pmmp,
        ):
            # ---- constants ----
            wt13 = constp.tile([128, NTAP, UNITS], BF16, tag="wt13")
            nc.sync.dma_start(out=wt13, in_=wt13_d.ap().rearrange("t k u -> k t u"))
            wt2p = constp.tile([128, 3, UNITS], BF16, tag="wt2p")
            nc.sync.dma_start(out=wt2p, in_=wt2p_d.ap().rearrange("t k u -> k t u"))
            wsl = constp.tile([64, 3, UNITS], BF16, tag="wsl")
            nc.sync.dma_start(out=wsl, in_=wsl_d.ap().rearrange("t k u -> k t u"))
            aexp = constp.tile([128, 1], F32, tag="aexp")
            nc.sync.dma_start(out=aexp, in_=aexp_d[:, :])

            # ---- big persistent buffers ----
            A1 = bigp.tile([128, HP * WP], BF16, tag="A1")
            A2 = bigp.tile([128, HP * WP], BF16, tag="A2")
            a1v = A1.rearrange("p (hp wp) -> p hp wp", wp=WP)
            a2v = A2.rearrange("p (hp wp) -> p hp wp", wp=WP)
            lbc = bigp.tile([128, HW], BF16, tag="lbc")

            # input image chunks: rows [0,34) [34,66) [66,98) [98,130)
            chunk_rows = [(0, 34), (34, 66), (66, 98), (98, 130)]
            for r0, r1 in chunk_rows:
                nc.sync.dma_start(
                    out=A1[:, r0 * WP : r1 * WP], in_=a1_d[:, r0 * WP : r1 * WP]
                )
                nc.sync.dma_start(
                    out=A2[:, r0 * WP : r1 * WP], in_=a2_d[:, r0 * WP : r1 * WP]
                )

            # ---- norm image: sum(x^2) -> 3x3 box -> ln -> broadcast ----
            s2p = bigp.tile([128, HP], BF16, tag="s2p")  # (w, padded h)
            nc.vector.memset(s2p[:, 0:1], 0.0)
            nc.vector.memset(s2p[:, HP - 1 : HP], 0.0)
            xwv = xw_d.ap().rearrange("p (h c) -> p h c", c=C)
            for s in range(NPSLAB):
                h0 = s * PSLAB
                xws = p1p.tile([128, PSLAB, C], BF16, tag="xws")
                nc.sync.dma_start(out=xws, in_=xwv[:, h0 : h0 + PSLAB])
                xsq = p1p.tile([128, PSLAB, C], BF16, tag="xsq")
                nc.vector.tensor_tensor(
                    out=xsq, in0=xws, in1=xws, op=ALU.mult
                )
                with nc.allow_low_precision(reason="s2 bf16 ~5e-4 rel; out err ~a*2.5e-4"):
                    nc.vector.tensor_reduce(
                        out=s2p[:, 1 + h0 : 1 + h0 + PSLAB],
                        in_=xsq,
                        axis=mybir.AxisListType.X,
                        op=ALU.add,
                    )
            timg = bigp.tile([128, 128], BF16, tag="timg")  # (w, h) h-boxed
            nc.vector.tensor_tensor(
                out=timg, in0=s2p[:, 0:128], in1=s2p[:, 1:129], op=ALU.add
            )
            nc.vector.tensor_tensor(
                out=timg, in0=timg, in1=s2p[:, 2:130], op=ALU.add
            )
            t2p = bigp.tile([128, WP], BF16, tag="t2p")  # (h, padded w)
            nc.vector.memset(t2p[:, 0:1], 0.0)
            nc.vector.memset(t2p[:, WP - 1 : WP], 0.0)
            nc.vector.transpose(out=t2p[:, 1:129], in_=timg)
            badd = bigp.tile([128, 128], BF16, tag="badd")  # (h, w) full box
            nc.vector.tensor_tensor(
                out=badd, in0=t2p[:, 0:128], in1=t2p[:, 1:129], op=ALU.add
            )
            nc.vector.tensor_tensor(
                out=badd, in0=badd, in1=t2p[:, 2:130], op=ALU.add
            )
            lrow = bigp.tile([128, 128], BF16, tag="lrow")  # (h, w) ln(box)
            nc.scalar.activation(out=lrow, in_=badd, func=AF.Ln)
            nc.sync.dma_start(out=lbc[0:1, :], in_=lrow)
            n = 1
            while n < 128:
                nc.sync.dma_start(out=lbc[n : 2 * n, :], in_=lbc[0:n, :])
                n *= 2

            # ---- GEMM per tile: 15 matmuls accumulate into one PSUM bank ----
            def emit_gemm_tile(ps, j):
                hh = j * TILE_ROWS
                first = True
                for ty in range(3):
                    for tx in range(3):
                        nc.tensor.matmul(
                            ps,
                            wt13[:, ty * 3 + tx, :],
                            a1v[:, hh + ty : hh + ty + TILE_ROWS, tx : tx + W],
                            start=first,
                            stop=False,
                        )
                        first = False
                for ty in range(3):
                    nc.tensor.matmul(
                        ps,
                        wt2p[:, ty, :],
                        a2v[:, hh + ty : hh + ty + TILE_ROWS, 0:W],
                        start=False,
                        stop=False,
                    )
                for ty in range(3):
                    nc.tensor.matmul(
                        ps,
                        wsl[:, ty, :],
                        a2v[0:64, hh + ty : hh + ty + TILE_ROWS, 2 : 2 + W],
                        start=False,
                        stop=(ty == 2),
                    )

            for b in range(NB):
                s16 = epip.tile([128, BPIX], BF16, tag="s16")
                for gi in range(BG):
                    g = b * BG + gi
                    psg = pmmp.tile([128, GT, NPIX], F32, tag="ps")
                    for j in range(GT):
                        emit_gemm_tile(psg[:, j, :], g * GT + j)
                    psf = psg.rearrange("p t n -> p (t n)")
                    half = slice(gi * GPIX, (gi + 1) * GPIX)
                    nc.vector.tensor_copy(out=s16[:, half], in_=psf)
                sq = epip.tile([128, BPIX], BF16, tag="sq")
                nc.vector.tensor_tensor(out=sq, in0=s16, in1=s16, op=ALU.mult)
                v = epip.tile([128, BPIX], BF16, tag="v", bufs=1)
                nc.scalar.activation(out=v, in_=sq, func=AF.Ln)
                # v2 = ln(s^2) - ln(box)
                v2 = epip.tile([128, BPIX], BF16, tag="v2", bufs=1)
                nc.gpsimd.tensor_tensor(
                    out=v2,
                    in0=v,
                    in1=lbc[:, b * BPIX : (b + 1) * BPIX],
                    op=ALU.subtract,
                )
                t3 = epip.tile([128, BPIX], BF16, tag="t3")
                nc.scalar.activation(out=t3, in_=v2, func=AF.Exp, scale=aexp[:, :])
                # sign(s)*t3 via saturating clamp: min(max(s*1e18, -1), 1) * t3
                c1 = epip.tile([128, BPIX], BF16, tag="c1", bufs=1)
                nc.vector.tensor_scalar(
                    out=c1,
                    in0=s16,
                    scalar1=1e18,
                    scalar2=-1.0,
                    op0=ALU.mult,
                    op1=ALU.max,
                )
                o = epip.tile([128, BPIX], BF16, tag="sq")
                nc.vector.scalar_tensor_tensor(
                    out=o,
                    in0=c1,
                    scalar=1.0,
                    in1=t3,
                    op0=ALU.min,
                    op1=ALU.mult,
                )
                nc.sync.dma_start(out=out_d[:, b * BPIX : (b + 1) * BPIX], in_=o)

    nc.compile()
    return nc


def _host_prep_w(w, p, q):
    EPS = 1e-12
    w64 = w[0].astype(np.float64)  # (576, 128)
    qv = (q.astype(np.float64) ** 2 / 10.0)[0]
    wn = np.sqrt(np.maximum((w64**2).sum(0), EPS)) + qv
    what = (w64 / wn).astype(np.float32)
    wh = what.astype(ml_dtypes.bfloat16)
    wl = (what - wh.astype(np.float32)).astype(ml_dtypes.bfloat16)

    def tap(a, k):
        return np.ascontiguousarray(a[k * 64 : (k + 1) * 64, :])

    wt13 = np.stack([np.vstack([tap(wh, k), tap(wh, k)]) for k in range(9)])
    wt2p = np.stack(
        [np.vstack([tap(wl, 3 * ty + 0), tap(wl, 3 * ty + 1)]) for ty in range(3)]
    )
    wsl = np.stack([tap(wl, 2), tap(wl, 5), tap(wl, 8)])

    pe = np.exp(p.astype(np.float64) - p.astype(np.float64).max())
    a = pe / pe.sum()
    aexp = (a * 0.5).astype(np.float32).reshape(128, 1)
    return wt13, wt2p, wsl, aexp


def _host_prep_x(x):
    # x: (H, W, C) fp32 -> A1, A2 (128, HP*WP) bf16 and XW (128, H*C) bf16
    xt = np.ascontiguousarray(x.reshape(HW, C).T)  # (C, HW) fp32
    xh = xt.astype(ml_dtypes.bfloat16)
    xl = (xt - xh.astype(np.float32)).astype(ml_dtypes.bfloat16)
    A1 = np.zeros((128, HP, WP), dtype=ml_dtypes.bfloat16)
    A1[0:C, 1 : 1 + H, 1 : 1 + W] = xh.reshape(C, H, W)
    A1[C : 2 * C, 1 : 1 + H, 1 : 1 + W] = xl.reshape(C, H, W)
    A2 = np.zeros((128, HP, WP), dtype=ml_dtypes.bfloat16)
    A2[0:C] = A1[0:C]
    A2[C : 2 * C, :, 0 : WP - 1] = A1[0:C, :, 1:WP]
    XW = np.ascontiguousarray(
        x.astype(ml_dtypes.bfloat16).transpose(1, 0, 2)
    )  # (W, H, C)
    return (
        A1.reshape(128, HP * WP),
        A2.reshape(128, HP * WP),
        XW.reshape(128, H * C),
    )


LAST_RESULTS = None


def kernel(inputs, w, p, q):
    global LAST_RESULTS
    if "nc" not in _CACHE:
        _CACHE["nc"] = _build()
    nc = _CACHE["nc"]

    wt13, wt2p, wsl, aexp = _host_prep_w(w, p, q)
    xs = inputs.astype(np.float32)
    in_maps = []
    for b in range(B):
        A1, A2, XW = _host_prep_x(xs[b])
        in_maps.append(
            {
                "a1": A1,
                "a2i": A2,
                "xw": XW,
                "wt13": wt13,
                "wt2p": wt2p,
                "wsl": wsl,
                "aexp": aexp,
            }
        )
    import os

    trace = bool(int(os.environ.get("KERNEL_TRACE", "0")))
    res = run_bass_kernel_spmd(nc, in_maps, core_ids=list(range(B)), trace=trace)
    LAST_RESULTS = res
    out = np.stack(
        [
            res.results[b]["out"].astype(np.float32).T.reshape(H, W, UNITS)
            for b in range(B)
        ]
    )
    return out


# revision 19
# speedup vs baseline: 1.4378x; 1.1177x over previous
"""Trainium2 Bass kernel for nn_CosSimConv2D.

Math (per sample b):
  s    = im2col3x3(x) @ w_hat           where w_hat = w / (||w||_col + qv)
  out  = sign(s) * exp(a_u/2 * (ln(s^2) - ln(box)))
  box  = 3x3 box-filter of per-pixel sum(x^2)  (= ||im2col row||^2)
  a    = softmax(p)
(The eps=1e-12 terms of the reference are dropped; they are ~1e-7-relative.)

GEMM precision: x and w_hat are each split hi+lo in bf16; three product
terms (xh@wh + xl@wh + xh@wl) recover ~fp32-grade dot products.
Data-parallel over batch: core b computes sample b.

Device layouts (host-prepared, DMA'd straight into SBUF):
  A1 (128p, 130*130) bf16 : p0-63  = x_hi^T zero-padded image,
                            p64-127 = x_lo^T zero-padded image
  A2 (128p, 130*130) bf16 : p0-63  = x_hi^T,
                            p64-127 = x_hi^T shifted +1 col
  XW (128p, 128*64)  bf16 : (w, h, c) layout of x_hi for the norm image
GEMM: 8 groups x 4 tiles x 15 matmuls (N=512) accumulate in a 4-bank
PSUM group tile (bufs=2 -> all 8 banks); epilogue per 2-group batch:
  sq = s*s (DVE), v = Ln(sq) (ACT), v2 = v - lnbox_bcast (DVE),
  t3 = Exp(a/2 * v2) (ACT), out = t3 | signbit(s) (DVE), DMA out bf16.
Host converts the (128u, HW) bf16 result to (H, W, UNITS) fp32.
"""

import sys

sys.path.insert(0, "/opt/trn_rl_repo")

import numpy as np
import ml_dtypes

import concourse.bass as bass
import concourse.mybir as mybir
import concourse.tile as tile
from concourse import bacc
from concourse.bass_utils import run_bass_kernel_spmd

BF16 = mybir.dt.bfloat16
F32 = mybir.dt.float32
AF = mybir.ActivationFunctionType
ALU = mybir.AluOpType

B, H, W, C, UNITS = 8, 128, 128, 64, 128
HW = H * W  # 16384
HP, WP = H + 2, W + 2  # 130x130 padded image
NTAP = 9
TILE_ROWS = 4  # image rows per output tile -> N = 512
NPIX = TILE_ROWS * W  # 512
GT = 4  # tiles per PSUM group (4 banks)
NG = 8  # groups
GPIX = GT * NPIX  # 2048
BG = 2  # groups per epilogue batch
NB = NG // BG  # 4 batches
BPIX = BG * GPIX  # 4096
PSLAB = 16  # image rows per norm-pass slab
NPSLAB = H // PSLAB  # 8

_CACHE = {}


def _build():
    nc = bacc.Bacc("TRN2", target_bir_lowering=False, debug=False)

    a1_d = nc.dram_tensor("a1", [128, HP * WP], BF16, kind="ExternalInput")
    a2_d = nc.dram_tensor("a2i", [128, HP * WP], BF16, kind="ExternalInput")
    xw_d = nc.dram_tensor("xw", [128, H * C], BF16, kind="ExternalInput")
    wt13_d = nc.dram_tensor("wt13", [NTAP, 128, UNITS], BF16, kind="ExternalInput")
    wt2p_d = nc.dram_tensor("wt2p", [3, 128, UNITS], BF16, kind="ExternalInput")
    wsl_d = nc.dram_tensor("wsl", [3, 64, UNITS], BF16, kind="ExternalInput")
    aexp_d = nc.dram_tensor("aexp", [128, 1], F32, kind="ExternalInput")
    out_d = nc.dram_tensor("out", [128, HW], BF16, kind="ExternalOutput")

    with tile.TileContext(nc) as tc:
        with (
            tc.tile_pool(name="const", bufs=1) as constp,
            tc.tile_pool(name="big", bufs=1) as bigp,
            tc.tile_pool(name="p1", bufs=2) as p1p,
            tc.tile_pool(name="epi", bufs=2) as epip,
            tc.tile_pool(name="pmm", bufs=2, space="PSUM") as pmmp,
        ):
            # ---- constants (issued first; tiny) ----
            wt13 = constp.tile([128, NTAP, UNITS], BF16, tag="wt13")
            nc.sync.dma_start(out=wt13, in_=wt13_d.ap().rearrange("t k u -> k t u"))
            wt2p = constp.tile([128, 3, UNITS], BF16, tag="wt2p")
            nc.sync.dma_start(out=wt2p, in_=wt2p_d.ap().rearrange("t k u -> k t u"))
            wsl = constp.tile([64, 3, UNITS], BF16, tag="wsl")
            nc.sync.dma_start(out=wsl, in_=wsl_d.ap().rearrange("t k u -> k t u"))
            aexp = constp.tile([128, 1], F32, tag="aexp")
            nc.sync.dma_start(out=aexp, in_=aexp_d[:, :])

            # ---- big persistent buffers ----
            A1 = bigp.tile([128, HP * WP], BF16, tag="A1")
            A2 = bigp.tile([128, HP * WP], BF16, tag="A2")
            a1v = A1.rearrange("p (hp wp) -> p hp wp", wp=WP)
            a2v = A2.rearrange("p (hp wp) -> p hp wp", wp=WP)
            lbc = bigp.tile([128, HW], BF16, tag="lbc")

            # input image chunks; A1 on sync queue, A2 on scalar queue so the
            # first-group chunks transfer in parallel
            chunk_rows = [(0, 18), (18, 34), (34, 66), (66, 98), (98, 130)]
            for r0, r1 in chunk_rows:
                nc.sync.dma_start(
                    out=A1[:, r0 * WP : r1 * WP], in_=a1_d[:, r0 * WP : r1 * WP]
                )
                nc.scalar.dma_start(
                    out=A2[:, r0 * WP : r1 * WP], in_=a2_d[:, r0 * WP : r1 * WP]
                )

            # ---- norm image: sum(x^2) -> 3x3 box -> ln -> broadcast ----
            s2p = bigp.tile([128, HP], BF16, tag="s2p")  # (w, padded h)
            nc.vector.memset(s2p[:, 0:1], 0.0)
            nc.vector.memset(s2p[:, HP - 1 : HP], 0.0)
            xwv = xw_d.ap().rearrange("p (h c) -> p h c", c=C)
            for s in range(NPSLAB):
                h0 = s * PSLAB
                xws = p1p.tile([128, PSLAB, C], BF16, tag="xws")
                nc.gpsimd.dma_start(out=xws, in_=xwv[:, h0 : h0 + PSLAB])
                xsq = p1p.tile([128, PSLAB, C], BF16, tag="xsq")
                nc.vector.tensor_tensor(
                    out=xsq, in0=xws, in1=xws, op=ALU.mult
                )
                with nc.allow_low_precision(reason="s2 bf16 ~5e-4 rel; out err ~a*2.5e-4"):
                    nc.vector.tensor_reduce(
                        out=s2p[:, 1 + h0 : 1 + h0 + PSLAB],
                        in_=xsq,
                        axis=mybir.AxisListType.X,
                        op=ALU.add,
                    )
            timg = bigp.tile([128, 128], BF16, tag="timg")  # (w, h) h-boxed
            nc.vector.tensor_tensor(
                out=timg, in0=s2p[:, 0:128], in1=s2p[:, 1:129], op=ALU.add
            )
            nc.vector.tensor_tensor(
                out=timg, in0=timg, in1=s2p[:, 2:130], op=ALU.add
            )
            t2p = bigp.tile([128, WP], BF16, tag="t2p")  # (h, padded w)
            nc.vector.memset(t2p[:, 0:1], 0.0)
            nc.vector.memset(t2p[:, WP - 1 : WP], 0.0)
            nc.vector.transpose(out=t2p[:, 1:129], in_=timg)
            badd = bigp.tile([128, 128], BF16, tag="badd")  # (h, w) full box
            nc.vector.tensor_tensor(
                out=badd, in0=t2p[:, 0:128], in1=t2p[:, 1:129], op=ALU.add
            )
            nc.vector.tensor_tensor(
                out=badd, in0=badd, in1=t2p[:, 2:130], op=ALU.add
            )
            lrow = bigp.tile([128, 128], BF16, tag="lrow")  # (h, w) ln(box)
            nc.scalar.activation(out=lrow, in_=badd, func=AF.Ln)
            nc.sync.dma_start(out=lbc[0:1, :], in_=lrow)
            # doubling broadcast; large steps split across 3 DMA-issue engines
            n = 1
            while n < 16:
                nc.sync.dma_start(out=lbc[n : 2 * n, :], in_=lbc[0:n, :])
                n *= 2
            while n < 128:
                k = n // 3
                e0, e1 = n + k, n + 2 * k
                nc.sync.dma_start(out=lbc[n:e0, :], in_=lbc[0 : e0 - n, :])
                nc.scalar.dma_start(out=lbc[e0:e1, :], in_=lbc[e0 - n : e1 - n, :])
                nc.gpsimd.dma_start(out=lbc[e1 : 2 * n, :], in_=lbc[e1 - n : n, :])
                n *= 2

            # ---- GEMM per tile: 15 matmuls accumulate into one PSUM bank ----
            def emit_gemm_tile(ps, j):
                hh = j * TILE_ROWS
                first = True
                for ty in range(3):
                    for tx in range(3):
                        nc.tensor.matmul(
                            ps,
                            wt13[:, ty * 3 + tx, :],
                            a1v[:, hh + ty : hh + ty + TILE_ROWS, tx : tx + W],
                            start=first,
                            stop=False,
                        )
                        first = False
                for ty in range(3):
                    nc.tensor.matmul(
                        ps,
                        wt2p[:, ty, :],
                        a2v[:, hh + ty : hh + ty + TILE_ROWS, 0:W],
                        start=False,
                        stop=False,
                    )
                for ty in range(3):
                    nc.tensor.matmul(
                        ps,
                        wsl[:, ty, :],
                        a2v[0:64, hh + ty : hh + ty + TILE_ROWS, 2 : 2 + W],
                        start=False,
                        stop=(ty == 2),
                    )

            # ---- software-pipelined epilogue over batches ----
            # stage1(b): GEMM + PSUM->bf16 casts + s^2 + Ln
            # stage2(b): v2 = v - lnbox, t3 = exp(a/2 * v2)
            # stage3(b): sign clamp + multiply + out DMA
            # Emission order per step: stage2(b-1), stage3(b-2), stage1(b) so
            # no engine queue head-blocks on a not-yet-computed dependency.
            st = {}

            def stage1(b):
                s16 = epip.tile([128, BPIX], BF16, tag="s16")
                for gi in range(BG):
                    g = b * BG + gi
                    psg = pmmp.tile([128, GT, NPIX], F32, tag="ps")
                    for j in range(GT):
                        emit_gemm_tile(psg[:, j, :], g * GT + j)
                    psf = psg.rearrange("p t n -> p (t n)")
                    half = slice(gi * GPIX, (gi + 1) * GPIX)
                    nc.vector.tensor_copy(out=s16[:, half], in_=psf)
                sq = epip.tile([128, BPIX], BF16, tag="sq")
                nc.vector.tensor_tensor(out=sq, in0=s16, in1=s16, op=ALU.mult)
                v = epip.tile([128, BPIX], BF16, tag="v", bufs=1)
                nc.scalar.activation(out=v, in_=sq, func=AF.Ln)
                st[b] = (s16, v)

            def stage2(b):
                s16, v = st[b]
                v2 = epip.tile([128, BPIX], BF16, tag="v2")
                nc.vector.tensor_tensor(
                    out=v2,
                    in0=v,
                    in1=lbc[:, b * BPIX : (b + 1) * BPIX],
                    op=ALU.subtract,
                )
                t3 = epip.tile([128, BPIX], BF16, tag="t3")
                nc.scalar.activation(out=t3, in_=v2, func=AF.Exp, scale=aexp[:, :])
                st[b] = (s16, t3)

            def stage3(b):
                s16, t3 = st.pop(b)
                # sign(s)*t3 via saturating clamp: min(max(s*1e18, -1), 1) * t3
                c1 = epip.tile([128, BPIX], BF16, tag="c1", bufs=1)
                nc.vector.tensor_scalar(
                    out=c1,
                    in0=s16,
                    scalar1=1e18,
                    scalar2=-1.0,
                    op0=ALU.mult,
                    op1=ALU.max,
                )
                c2 = epip.tile([128, BPIX], BF16, tag="c2", bufs=1)
                nc.vector.tensor_scalar(
                    out=c2,
                    in0=c1,
                    scalar1=1.0,
                    scalar2=None,
                    op0=ALU.min,
                )
                o = epip.tile([128, BPIX], BF16, tag="sq")
                nc.vector.tensor_tensor(out=o, in0=c2, in1=t3, op=ALU.mult)
                nc.sync.dma_start(out=out_d[:, b * BPIX : (b + 1) * BPIX], in_=o)

            for step in range(NB + 2):
                if 1 <= step <= NB:
                    stage2(step - 1)
                if step >= 2:
                    stage3(step - 2)
                if step < NB:
                    stage1(step)

    nc.compile()
    return nc


def _host_prep_w(w, p, q):
    EPS = 1e-12
    w64 = w[0].astype(np.float64)  # (576, 128)
    qv = (q.astype(np.float64) ** 2 / 10.0)[0]
    wn = np.sqrt(np.maximum((w64**2).sum(0), EPS)) + qv
    what = (w64 / wn).astype(np.float32)
    wh = what.astype(ml_dtypes.bfloat16)
    wl = (what - wh.astype(np.float32)).astype(ml_dtypes.bfloat16)

    def tap(a, k):
        return np.ascontiguousarray(a[k * 64 : (k + 1) * 64, :])

    wt13 = np.stack([np.vstack([tap(wh, k), tap(wh, k)]) for k in range(9)])
    wt2p = np.stack(
        [np.vstack([tap(wl, 3 * ty + 0), tap(wl, 3 * ty + 1)]) for ty in range(3)]
    )
    wsl = np.stack([tap(wl, 2), tap(wl, 5), tap(wl, 8)])

    pe = np.exp(p.astype(np.float64) - p.astype(np.float64).max())
    a = pe / pe.sum()
    aexp = (a * 0.5).astype(np.float32).reshape(128, 1)
    return wt13, wt2p, wsl, aexp


def _host_prep_x(x):
    # x: (H, W, C) fp32 -> A1, A2 (128, HP*WP) bf16 and XW (128, H*C) bf16
    xt = np.ascontiguousarray(x.reshape(HW, C).T)  # (C, HW) fp32
    xh = xt.astype(ml_dtypes.bfloat16)
    xl = (xt - xh.astype(np.float32)).astype(ml_dtypes.bfloat16)
    A1 = np.zeros((128, HP, WP), dtype=ml_dtypes.bfloat16)
    A1[0:C, 1 : 1 + H, 1 : 1 + W] = xh.reshape(C, H, W)
    A1[C : 2 * C, 1 : 1 + H, 1 : 1 + W] = xl.reshape(C, H, W)
    A2 = np.zeros((128, HP, WP), dtype=ml_dtypes.bfloat16)
    A2[0:C] = A1[0:C]
    A2[C : 2 * C, :, 0 : WP - 1] = A1[0:C, :, 1:WP]
    XW = np.ascontiguousarray(
        x.astype(ml_dtypes.bfloat16).transpose(1, 0, 2)
    )  # (W, H, C)
    return (
        A1.reshape(128, HP * WP),
        A2.reshape(128, HP * WP),
        XW.reshape(128, H * C),
    )


LAST_RESULTS = None


def kernel(inputs, w, p, q):
    global LAST_RESULTS
    if "nc" not in _CACHE:
        _CACHE["nc"] = _build()
    nc = _CACHE["nc"]

    wt13, wt2p, wsl, aexp = _host_prep_w(w, p, q)
    xs = inputs.astype(np.float32)
    in_maps = []
    for b in range(B):
        A1, A2, XW = _host_prep_x(xs[b])
        in_maps.append(
            {
                "a1": A1,
                "a2i": A2,
                "xw": XW,
                "wt13": wt13,
                "wt2p": wt2p,
                "wsl": wsl,
                "aexp": aexp,
            }
        )
    import os

    trace = bool(int(os.environ.get("KERNEL_TRACE", "0")))
    res = run_bass_kernel_spmd(nc, in_maps, core_ids=list(range(B)), trace=trace)
    LAST_RESULTS = res
    out = np.stack(
        [
            res.results[b]["out"].astype(np.float32).T.reshape(H, W, UNITS)
            for b in range(B)
        ]
    )
    return out
